# revision 7
# baseline (speedup 1.0000x reference)
"""Triangular pairwise channel product on 8 Trainium2 NeuronCores.

out[b,h,w,k] = x[b,h,w,i_k] * x[b,h,w,j_k]  for the C*(C-1)/2 pairs
(i<j) in row-major (np.triu_indices) order.

Sharding: pure data parallel over batch — core c takes x[2c:2c+2].
Per core the 2*64*64 = 8192 spatial positions map to 128 SBUF
partitions (b_loc*64+h) x 64 groups (w).  For each group-block of G=8
positions, block i of the output (pairs (i, i+1..63)) is one fp32
tensor_tensor multiply whose first operand is x[:, :, i] broadcast via
a step-0 access pattern — 63 DVE ops cover all 2016 output channels
with per-partition-contiguous stores back to HBM.
"""

import numpy as np

import concourse.bacc as bacc
import concourse.bass as bass
import concourse.mybir as mybir
import concourse.tile as tile
from concourse.bass_utils import run_bass_kernel_spmd

B, H, W, C = 16, 64, 64, 64
K = C * (C - 1) // 2  # 2016
N_CORES = 8
BP = B // N_CORES  # batch rows per core
P = BP * H         # 128 SBUF partitions
G_TOTAL = W        # position groups per partition
G = 8              # groups processed per iteration
GP_SPLIT = 12      # blocks [0, GP_SPLIT) computed on GPSIMD, rest on DVE
FP = mybir.dt.float32

_nc_cache = None


def build_bass() -> bass.Bass:
    # Bacc (not plain Bass): its compile() pipeline runs
    # generate_event_semaphores, which splits multi-wait instructions to
    # satisfy the TRN2 1-wait-per-instruction codegen limit.
    nc = bacc.Bacc(
        "TRN2",
        target_bir_lowering=False,
        debug=False,
        num_devices=N_CORES,
    )
    x = nc.dram_tensor("x", [P, G_TOTAL, C], FP, kind="ExternalInput")
    y = nc.dram_tensor("y", [P, G_TOTAL, K], FP, kind="ExternalOutput")

    with tile.TileContext(nc) as tc:
        with (
            tc.tile_pool(name="xin", bufs=4) as xpool,
            tc.tile_pool(name="out", bufs=2) as opool,
        ):
            for it in range(G_TOTAL // G):
                xt = xpool.tile([P, G, C], FP)
                # Input loads ride the ACT HWDGE ring so they never queue
                # behind the 8 MB output stores on the SP ring.
                nc.scalar.dma_start(out=xt[:], in_=x[:, it * G : (it + 1) * G, :])

                ot = opool.tile([P, G, K], FP)
                ro = 0
                for i in range(C - 1):
                    w = C - 1 - i
                    a = xt[:, :, i : i + 1].broadcast_to([P, G, w])
                    b = xt[:, :, i + 1 : C]
                    # Split compute: the widest blocks go to GPSIMD (few
                    # ops, so its higher dispatch cost amortizes), the
                    # long tail of narrow blocks stays on the DVE.
                    eng = nc.gpsimd if i < GP_SPLIT else nc.vector
                    eng.tensor_mul(ot[:, :, ro : ro + w], a, b)
                    ro += w

                nc.sync.dma_start(out=y[:, it * G : (it + 1) * G, :], in_=ot[:])

    nc.finalize()
    return nc


def make_in_maps(x: np.ndarray) -> list[dict[str, np.ndarray]]:
    x = np.ascontiguousarray(x, dtype=np.float32)
    return [
        {"x": x[c * BP : (c + 1) * BP].reshape(P, G_TOTAL, C)} for c in range(N_CORES)
    ]


def kernel(**inputs: np.ndarray) -> np.ndarray:
    global _nc_cache
    if _nc_cache is None:
        _nc_cache = build_bass()
    res = run_bass_kernel_spmd(
        _nc_cache, make_in_maps(inputs["inputs"]), list(range(N_CORES))
    ).results
    return np.concatenate(
        [res[c]["y"].reshape(BP, H, W, K) for c in range(N_CORES)], axis=0
    )


# revision 9
# speedup vs baseline: 1.0915x; 1.0915x over previous
"""Triangular pairwise channel product on 8 Trainium2 NeuronCores.

out[b,h,w,k] = x[b,h,w,i_k] * x[b,h,w,j_k]  for the C*(C-1)/2 pairs
(i<j) in row-major (np.triu_indices) order.

Sharding: pure data parallel over batch — core c takes x[2c:2c+2].
Per core the 2*64*64 = 8192 spatial positions map to 128 SBUF
partitions (b_loc*64+h) x 64 groups (w).  For each group-block of G=8
positions, block i of the output (pairs (i, i+1..63)) is one fp32
tensor_tensor multiply whose first operand is x[:, :, i] broadcast via
a step-0 access pattern — 63 DVE ops cover all 2016 output channels
with per-partition-contiguous stores back to HBM.
"""

import numpy as np

import concourse.bacc as bacc
import concourse.bass as bass
import concourse.mybir as mybir
import concourse.tile as tile
from concourse.bass_utils import run_bass_kernel_spmd

B, H, W, C = 16, 64, 64, 64
K = C * (C - 1) // 2  # 2016
N_CORES = 8
BP = B // N_CORES  # batch rows per core
P = BP * H         # 128 SBUF partitions
G_TOTAL = W        # position groups per partition
G = 8              # groups processed per iteration
SPLIT_BLK = 40     # blocks [0, SPLIT_BLK) -> big piece, rest -> small piece
FP = mybir.dt.float32

_row = [0]
for _i in range(C):
    _row.append(_row[-1] + C - 1 - _i)
KA = _row[SPLIT_BLK]  # channels in the big piece
KB = K - KA           # channels in the small piece

_nc_cache = None


def build_bass() -> bass.Bass:
    # Bacc (not plain Bass): its compile() pipeline runs
    # generate_event_semaphores, which splits multi-wait instructions to
    # satisfy the TRN2 1-wait-per-instruction codegen limit.
    nc = bacc.Bacc(
        "TRN2",
        target_bir_lowering=False,
        debug=False,
        num_devices=N_CORES,
    )
    x = nc.dram_tensor("x", [P, G_TOTAL, C], FP, kind="ExternalInput")
    y = nc.dram_tensor("y", [P, G_TOTAL, K], FP, kind="ExternalOutput")

    with tile.TileContext(nc) as tc:
        with (
            tc.tile_pool(name="xin", bufs=1) as xpool,
            tc.tile_pool(name="oa", bufs=2) as apool,
            tc.tile_pool(name="ob", bufs=2) as bpool,
        ):
            # The whole per-core input is 16 KB/partition — preload once.
            xt = xpool.tile([P, G_TOTAL, C], FP)
            nc.scalar.dma_start(out=xt[:], in_=x[:])

            for it in range(G_TOTAL // G):
                xg = xt[:, it * G : (it + 1) * G, :]
                # Each iteration's output is written as two tiles on two
                # HWDGE rings: the big piece (SP ring) starts draining
                # while the DVE finishes the small piece, and the final
                # iteration's tail drain is only the small piece.
                ota = apool.tile([P, G, KA], FP)
                otb = bpool.tile([P, G, KB], FP)
                for i in range(C - 1):
                    w = C - 1 - i
                    a = xg[:, :, i : i + 1].broadcast_to([P, G, w])
                    b = xg[:, :, i + 1 : C]
                    if i < SPLIT_BLK:
                        dst = ota[:, :, _row[i] : _row[i] + w]
                    else:
                        dst = otb[:, :, _row[i] - KA : _row[i] - KA + w]
                    nc.vector.tensor_mul(dst, a, b)

                nc.sync.dma_start(
                    out=y[:, it * G : (it + 1) * G, 0:KA], in_=ota[:]
                )
                nc.scalar.dma_start(
                    out=y[:, it * G : (it + 1) * G, KA:K], in_=otb[:]
                )

    nc.finalize()
    return nc


def make_in_maps(x: np.ndarray) -> list[dict[str, np.ndarray]]:
    x = np.ascontiguousarray(x, dtype=np.float32)
    return [
        {"x": x[c * BP : (c + 1) * BP].reshape(P, G_TOTAL, C)} for c in range(N_CORES)
    ]


def kernel(**inputs: np.ndarray) -> np.ndarray:
    global _nc_cache
    if _nc_cache is None:
        _nc_cache = build_bass()
    res = run_bass_kernel_spmd(
        _nc_cache, make_in_maps(inputs["inputs"]), list(range(N_CORES))
    ).results
    return np.concatenate(
        [res[c]["y"].reshape(BP, H, W, K) for c in range(N_CORES)], axis=0
    )


# revision 11
# speedup vs baseline: 1.2056x; 1.1045x over previous
"""Triangular pairwise channel product on 8 Trainium2 NeuronCores.

out[b,h,w,k] = x[b,h,w,i_k] * x[b,h,w,j_k]  for the C*(C-1)/2 pairs
(i<j) in row-major (np.triu_indices) order.

Sharding: pure data parallel over batch — core c takes x[2c:2c+2].
Per core the 2*64*64 = 8192 spatial positions map to 128 SBUF
partitions (b_loc*64+h) x 64 groups (w).  For each group-block of G=8
positions, block i of the output (pairs (i, i+1..63)) is one fp32
tensor_tensor multiply whose first operand is x[:, :, i] broadcast via
a step-0 access pattern — 63 DVE ops cover all 2016 output channels
with per-partition-contiguous stores back to HBM.
"""

import numpy as np

import concourse.bacc as bacc
import concourse.bass as bass
import concourse.mybir as mybir
import concourse.tile as tile
from concourse.bass_utils import run_bass_kernel_spmd

B, H, W, C = 16, 64, 64, 64
K = C * (C - 1) // 2  # 2016
N_CORES = 8
BP = B // N_CORES  # batch rows per core
P = BP * H         # 128 SBUF partitions
G_TOTAL = W        # position groups per partition
# Iteration group sizes: steady G=8, then a shrinking tail so the final
# output drain after the last compute op is only ~1 MB instead of ~8 MB.
G_ITERS = [8, 8, 8, 8, 8, 8, 8, 7, 1]
assert sum(G_ITERS) == W
G0 = G_ITERS[0]
FP = mybir.dt.float32

_row = [0]
for _i in range(C):
    _row.append(_row[-1] + C - 1 - _i)

_nc_cache = None


def build_bass() -> bass.Bass:
    # Bacc (not plain Bass): its compile() pipeline runs
    # generate_event_semaphores, which splits multi-wait instructions to
    # satisfy the TRN2 1-wait-per-instruction codegen limit.
    nc = bacc.Bacc(
        "TRN2",
        target_bir_lowering=False,
        debug=False,
        num_devices=N_CORES,
    )
    x = nc.dram_tensor("x", [P, G_TOTAL, C], FP, kind="ExternalInput")
    y = nc.dram_tensor("y", [P, G_TOTAL, K], FP, kind="ExternalOutput")

    with tile.TileContext(nc) as tc:
        with (
            tc.tile_pool(name="xin", bufs=1) as xpool,
            tc.tile_pool(name="out", bufs=2) as opool,
        ):
            # Preload the input in two pieces on the ACT ring: the first
            # iteration's groups land in ~2 us so compute starts early;
            # the rest (56 groups) streams in behind iteration 0.
            xt0 = xpool.tile([P, G0, C], FP, tag="x0")
            nc.scalar.dma_start(out=xt0[:], in_=x[:, 0:G0, :])
            xtr = xpool.tile([P, G_TOTAL - G0, C], FP, tag="xr")
            nc.scalar.dma_start(out=xtr[:], in_=x[:, G0:, :])

            g_off = 0
            for it, Gi in enumerate(G_ITERS):
                if it == 0:
                    xg = xt0[:, :, :]
                else:
                    xg = xtr[:, g_off - G0 : g_off - G0 + Gi, :]

                # All output stores ride the SP ring with full 2016-channel
                # rows (contiguous per-partition DRAM runs).
                ot = opool.tile([P, Gi, K], FP, tag="ot")
                for i in range(C - 1):
                    w = C - 1 - i
                    a = xg[:, :, i : i + 1].broadcast_to([P, Gi, w])
                    b = xg[:, :, i + 1 : C]
                    nc.vector.tensor_mul(ot[:, :, _row[i] : _row[i] + w], a, b)

                nc.sync.dma_start(out=y[:, g_off : g_off + Gi, :], in_=ot[:])
                g_off += Gi

    nc.finalize()
    return nc


def make_in_maps(x: np.ndarray) -> list[dict[str, np.ndarray]]:
    x = np.ascontiguousarray(x, dtype=np.float32)
    return [
        {"x": x[c * BP : (c + 1) * BP].reshape(P, G_TOTAL, C)} for c in range(N_CORES)
    ]


def kernel(**inputs: np.ndarray) -> np.ndarray:
    global _nc_cache
    if _nc_cache is None:
        _nc_cache = build_bass()
    res = run_bass_kernel_spmd(
        _nc_cache, make_in_maps(inputs["inputs"]), list(range(N_CORES))
    ).results
    return np.concatenate(
        [res[c]["y"].reshape(BP, H, W, K) for c in range(N_CORES)], axis=0
    )
